# revision 2
# baseline (speedup 1.0000x reference)
"""GIN message-passing classifier on 8 Trainium2 NeuronCores (v3).

Sharding: nodes (and incident edges, partitioned by dst) split contiguously
across 8 cores. Key design vs v2:
  - h table is PAIRED single-bf16: row r = [h[2r] | h[2r+1]] (128 cols,
    256B rows), so the AllGather moves 6.4MB (not 12.8) and indices fit
    int16 with no lo/hi table split.
  - edges are split host-side into even/odd src-parity streams; every
    128-edge chunk is parity-homogeneous, so scatter is ONE matmul per
    chunk (lhsT = gathered cols 0:64 or 64:128).
  - self term (1+eps)*h via identity matmul accumulated into the same
    PSUM as the aggregation (no DVE adds, no feature-major h copy).
  - MLP runs per 512-col chunk interleaved with aggregation; BN stats
    accumulate per chunk; BN apply is one ACT op producing bf16 directly.
  - iter 0 aggregation = embsT @ CNT dense matmuls (CNT edge counts
    precomputed on host).
  - BN stats AllReduced; h table AllGathered (compact).
Host sums per-core partial logits at unshard.
"""

import sys

sys.path.insert(0, "/opt/trn_rl_repo")

import numpy as np

import concourse.bass as bass
import concourse.bacc as bacc
import concourse.mybir as mybir
import concourse.tile as tile
from concourse import bass_utils, library_config

F32 = mybir.dt.float32
BF16 = mybir.dt.bfloat16
I16 = mybir.dt.int16
AX = mybir.AxisListType.X
ALU = mybir.AluOpType
ACT_F = mybir.ActivationFunctionType

OH_BATCH = 8   # one-hot chunks built per DVE op
OH_MODE = "tt"  # "tt": batched tensor_tensor; "ts": per-chunk tensor_scalar
SINGLE_PACKET = False  # dma_gather single_packet flag
OH_PREBUILD = 0  # eagerly emit oh builds for the next N groups

CFG_FULL = dict(
    NNODES=50000,
    NEDGES=1600000,
    D=64,
    NGRAPH=512,
    NCLS=53,
    VOCAB=3100,
    P=8,
    ITERS=3,
    WINSZ=128,
    GWIN=2,
    BN_EPS=1e-5,
    MLPCH=4,  # windows per MLP chunk
)


def _derive(cfg):
    c = dict(cfg)
    c["NPC"] = c["NNODES"] // c["P"]
    nwr = -(-c["NPC"] // c["WINSZ"])  # ceil
    c["NWIN"] = -(-nwr // c["GWIN"]) * c["GWIN"]
    c["NGRP"] = c["NWIN"] // c["GWIN"]
    c["NPAD"] = c["NWIN"] * c["WINSZ"]
    c["VP"] = -(-c["VOCAB"] // 128)
    return c


def _wrap16(idx):
    n = len(idx)
    assert n % 16 == 0
    arr = np.zeros((16, n // 16), np.int16)
    ar = np.arange(n)
    arr[ar % 16, ar // 16] = idx.astype(np.int16)
    return np.tile(arr, (8, 1))


def _bc_mats(cfg, gids, bc):
    P, NPC, NCLS, NG = cfg["P"], cfg["NPC"], cfg["NCLS"], cfg["NGRAPH"]
    bc = np.asarray(bc, np.float32)
    g0s = [int(gids[k * NPC]) for k in range(P)]
    first = np.searchsorted(gids, np.arange(NG), "left")
    owner = np.minimum(first // NPC, P - 1)
    mats = [np.zeros((NCLS, 128), np.float32) for _ in range(P)]
    for g in range(NG):
        k = int(owner[g])
        s = g - g0s[k]
        if 0 <= s < 128:
            mats[k][:, s] = bc
    return mats


def _prep(cfg, pkt, src, dst, gids, emb, eps, W1, b1, W2, b2, W3, b3, gamma, beta, Wc, bc):
    import ml_dtypes
    P, NPC, WINSZ, NWIN, GWIN, NGRP, D, VP = (
        cfg["P"], cfg["NPC"], cfg["WINSZ"], cfg["NWIN"], cfg["GWIN"],
        cfg["NGRP"], cfg["D"], cfg["VP"],
    )
    NPAD = cfg["NPAD"]
    pkt = np.asarray(pkt); src = np.asarray(src); dst = np.asarray(dst)
    gids = np.asarray(gids)

    k_of = dst // NPC
    per_core = []
    for k in range(P):
        m = k_of == k
        es = src[m]
        el = dst[m] - k * NPC
        win = el // WINSZ
        off = el % WINSZ
        par = (es % 2).astype(np.int64)
        per_core.append((es, win, off, par))

    # per-group static capacities per parity stream (max over cores/windows)
    cape = np.ones(NGRP, np.int64)
    capo = np.ones(NGRP, np.int64)
    for es, win, off, par in per_core:
        for h, caps in ((0, cape), (1, capo)):
            cnt = np.bincount(win[par == h], minlength=NWIN)
            wc_ = -(-cnt // 128)
            gc = wc_.reshape(NGRP, GWIN).max(axis=1)
            np.maximum(caps, gc, out=caps)
    caps_key = (tuple(int(x) for x in cape), tuple(int(x) for x in capo))

    ne_g = GWIN * cape * 128
    no_g = GWIN * capo * 128
    e_base = np.concatenate([[0], np.cumsum(ne_g)]).astype(int)
    o_base = np.concatenate([[0], np.cumsum(no_g)]).astype(int)
    ncols_g = GWIN * (cape + capo)
    col_base = np.concatenate([[0], np.cumsum(ncols_g)]).astype(int)
    NCHUNK = int(col_base[-1])
    NCHUNKB = -(-NCHUNK // OH_BATCH) * OH_BATCH

    emb = np.asarray(emb, np.float32)
    embh = emb.astype(ml_dtypes.bfloat16)  # [VOCAB, D]
    embs_pad = np.zeros((cfg["VOCAB"], 2 * D), ml_dtypes.bfloat16)
    embs_pad[:, 0:D] = embh
    VOCABP = VP * 128
    embsT_full = np.zeros((VOCABP, D), ml_dtypes.bfloat16)
    embsT_full[:cfg["VOCAB"]] = embh
    embsT = np.ascontiguousarray(
        embsT_full.reshape(VP, 128, D).transpose(1, 0, 2))  # [128, VP, D]

    bcm_all = _bc_mats(cfg, gids, bc)
    g0s = []
    in_maps = []
    for k in range(P):
        es, win, off, par = per_core[k]

        i23 = {}
        dso = np.full((NCHUNKB, 128), -1e6, np.float32)
        for h in (0, 1):
            sel = par == h
            w_h, off_h, es_h = win[sel], off[sel], es[sel]
            # sort by (window, src) so each 128-slot chunk reads ascending
            # table addresses (HBM row-buffer locality for the gather)
            order = np.lexsort((es_h, w_h))
            w_h, off_h, es_h = w_h[order], off_h[order], es_h[order]
            cnt = np.bincount(w_h, minlength=NWIN)
            start = np.concatenate([[0], np.cumsum(cnt)])[:-1]
            rank = np.arange(len(w_h)) - start[w_h]
            caps = (cape if h == 0 else capo)
            g = w_h // GWIN
            jw = w_h % GWIN
            capw = caps[g]
            base = (e_base if h == 0 else o_base)[g]
            pos = base + jw * (capw * 128) + rank
            size = int((e_base if h == 0 else o_base)[-1])
            idxs = np.zeros(size, np.int64)
            idxs[pos] = es_h // 2
            i23[h] = idxs
            cb = col_base[g] + (0 if h == 0 else GWIN * cape[g])
            col = cb + jw * capw + rank // 128
            dso[col, rank % 128] = off_h.astype(np.float32)

        def blocks(stream, base_arr):
            out = []
            for g in range(NGRP):
                out.append(_wrap16(stream[base_arr[g]:base_arr[g + 1]]))
            return np.concatenate(out, axis=1)

        i23e = blocks(i23[0], e_base)
        i23o = blocks(i23[1], o_base)

        # CNT matrix for iteration-0 dense aggregation
        m = k_of == k
        cnt0 = np.zeros((VOCABP, NPAD), np.float32)
        np.add.at(cnt0, (pkt[np.asarray(src[m])], np.asarray(dst[m]) - k * NPC), 1.0)
        cntT = np.ascontiguousarray(
            cnt0.reshape(VP, 128, NWIN, 128).transpose(1, 2, 0, 3)
        ).astype(ml_dtypes.bfloat16)  # [128, NWIN, VP, 128]

        nloc = np.zeros(NPAD, np.int64)
        nloc[:NPC] = pkt[k * NPC:(k + 1) * NPC]
        pktloc = _wrap16(nloc)

        g0 = int(gids[k * NPC])
        g0s.append(g0)
        gl = gids[k * NPC:(k + 1) * NPC] - g0
        assert gl.max() < 128, "graph span per core exceeds 128 slots"
        gw = np.full(NPAD, -1e6, np.float32)
        gw[:NPC] = gl.astype(np.float32)
        goff = gw.reshape(NWIN, 128).T.copy()  # [128, NWIN]

        consts = np.zeros((128, 640), np.float32)
        consts[:, 0:512] = np.tile(
            np.arange(128, dtype=np.float32)[None, :], (128, 4))
        consts[:, 512:640] = np.eye(128, dtype=np.float32)

        im = {
            "i23e": i23e, "i23o": i23o,
            "pktloc": pktloc,
            "dso": np.ascontiguousarray(dso.T),  # [128, NCHUNKB] f32
            "goff": goff,
            "embs": embs_pad,
            "embsT": embsT,
            "cntT": cntT,
            "Wmlp": np.stack([np.asarray(W1), np.asarray(W2), np.asarray(W3)], 1)
            .astype(np.float32),
            "bpack": np.stack(
                [np.asarray(b1), np.asarray(b2), np.asarray(b3),
                 np.asarray(gamma), np.asarray(beta),
                 np.full(D, 1.0 + float(np.asarray(eps)), np.float32),
                 np.full(D, cfg["BN_EPS"], np.float32),
                 np.zeros(D, np.float32)], 1
            ).astype(np.float32),
            "WcT": np.asarray(Wc, np.float32)
            .reshape(cfg["ITERS"], D, cfg["NCLS"]).transpose(1, 0, 2).copy(),
            "bcmat": bcm_all[k],
            "consts": consts,
        }
        in_maps.append(im)
    return in_maps, caps_key, g0s


def _build(cfg, caps, reps=1, skip=()):
    """skip: subset of {"agg","coll","cnt","oh","mlp"} -- timing ablations
    (results become incorrect; used only to attribute HW time)."""
    C = cfg
    cape, capo = caps
    D, NWIN, GWIN, NGRP, NPC, NPAD, VP = (
        C["D"], C["NWIN"], C["GWIN"], C["NGRP"], C["NPC"], C["NPAD"], C["VP"])
    ITERS = C["ITERS"]
    NCLS = C["NCLS"]
    MLPCH = C["MLPCH"]
    ne_g = [GWIN * cape[g] * 128 for g in range(NGRP)]
    no_g = [GWIN * capo[g] * 128 for g in range(NGRP)]
    e_base = np.concatenate([[0], np.cumsum(ne_g)]).astype(int)
    o_base = np.concatenate([[0], np.cumsum(no_g)]).astype(int)
    ncols_g = [GWIN * (cape[g] + capo[g]) for g in range(NGRP)]
    col_base = np.concatenate([[0], np.cumsum(ncols_g)]).astype(int)
    NCHUNK = int(col_base[-1])
    NCHUNKB = -(-NCHUNK // OH_BATCH) * OH_BATCH

    NSTAT = -(-NWIN // MLPCH)  # MLP/stat chunks per iter

    nc = bacc.Bacc(None, target_bir_lowering=False, debug=False,
                   num_swdge_queues=4)

    i23e = nc.dram_tensor("i23e", [128, int(e_base[-1]) // 16], I16, kind="ExternalInput")
    i23o = nc.dram_tensor("i23o", [128, int(o_base[-1]) // 16], I16, kind="ExternalInput")
    pktloc = nc.dram_tensor("pktloc", [128, NPAD // 16], I16, kind="ExternalInput")
    dso_d = nc.dram_tensor("dso", [128, NCHUNKB], F32, kind="ExternalInput")
    goff = nc.dram_tensor("goff", [128, NWIN], F32, kind="ExternalInput")
    embs_d = nc.dram_tensor("embs", [C["VOCAB"], 2 * D], BF16, kind="ExternalInput")
    embsT_d = nc.dram_tensor("embsT", [128, VP, D], BF16, kind="ExternalInput")
    cntT_d = nc.dram_tensor("cntT", [128, NWIN, VP, 128], BF16, kind="ExternalInput")
    Wmlp = nc.dram_tensor("Wmlp", [D, 3, D], F32, kind="ExternalInput")
    bpack = nc.dram_tensor("bpack", [D, 8], F32, kind="ExternalInput")
    WcT = nc.dram_tensor("WcT", [D, ITERS, NCLS], F32, kind="ExternalInput")
    bcmat = nc.dram_tensor("bcmat", [NCLS, 128], F32, kind="ExternalInput")
    consts = nc.dram_tensor("consts", [128, 640], F32, kind="ExternalInput")
    out = nc.dram_tensor("logits", [NCLS, 128], F32, kind="ExternalOutput")

    rg = [list(range(C["P"]))]

    with tile.TileContext(nc) as tc:
        with (
            tc.tile_pool(name="const", bufs=1) as cp,
            tc.tile_pool(name="x", bufs=1) as hx,
            tc.tile_pool(name="ge", bufs=2) as gep,
            tc.tile_pool(name="go", bufs=2) as gop,
            tc.tile_pool(name="ix", bufs=2) as ixp,
            tc.tile_pool(name="cnt", bufs=2) as cntp,
            tc.tile_pool(name="oh", bufs=6 + 8 * OH_PREBUILD) as ohp,
            tc.tile_pool(name="nm", bufs=1) as nmp,
            tc.tile_pool(name="small", bufs=1) as sp,
            tc.tile_pool(name="aggps", bufs=4, space="PSUM") as aggps,
            tc.tile_pool(name="mlpps", bufs=2, space="PSUM") as mlpps,
            tc.tile_pool(name="tpps", bufs=1, space="PSUM") as tpps,
            tc.tile_pool(name="plps", bufs=1, space="PSUM") as plps,
            tc.tile_pool(name="dram", bufs=2, space="DRAM") as dram,
        ):
            nc.gpsimd.load_library(library_config.mlp)

            for rep in range(reps):
                # resident constants
                cst = cp.tile([128, 640], F32, tag="cst")
                nc.sync.dma_start(cst[:], consts[:])
                iota4f = cst[:, 0:512]
                id128 = cst[:, 512:640]
                dso = cp.tile([128, NCHUNKB], F32, tag="dso")
                nc.sync.dma_start(dso[:], dso_d[:])
                iotb = cp.tile([128, OH_BATCH * 128], BF16, tag="iotb")
                for i in range(OH_BATCH // 4):
                    nc.vector.tensor_copy(iotb[:, i * 512:(i + 1) * 512], iota4f)
                iotbB = iotb[:].rearrange("p (c j) -> p c j", j=128)
                dsob = cp.tile([128, NCHUNKB], BF16, tag="dsob")
                nc.vector.tensor_copy(dsob[:], dso[:])
                gof = cp.tile([128, NWIN], F32, tag="gof")
                nc.sync.dma_start(gof[:], goff[:])
                emt = cp.tile([128, VP, D], BF16, tag="emt")
                nc.sync.dma_start(emt[:], embsT_d[:])
                wm = cp.tile([D, 3, D], F32, tag="wm")
                nc.sync.dma_start(wm[:], Wmlp[:])
                bp = cp.tile([D, 8], F32, tag="bp")
                nc.sync.dma_start(bp[:], bpack[:])
                wc = cp.tile([D, ITERS, NCLS], F32, tag="wc")
                nc.sync.dma_start(wc[:], WcT[:])
                bcm = cp.tile([NCLS, 128], F32, tag="bcm")
                nc.sync.dma_start(bcm[:], bcmat[:])
                # (1+eps) * I as bf16 for the self-term matmul; (1+eps) is
                # replicated to all 128 partitions via two DMA copies.
                ep128 = sp.tile([128, 1], F32, tag="ep128")
                nc.sync.dma_start(ep128[0:D, :], bpack[:, 5:6])
                nc.sync.dma_start(ep128[D:128, :], bpack[0:128 - D, 5:6])
                epsI = cp.tile([128, 128], BF16, tag="epsI")
                nc.vector.tensor_scalar(epsI[:], id128, ep128[:, 0:1], None,
                                        ALU.mult)

                xA = hx.tile([D, NPAD], F32, tag="xA")
                xB = hx.tile([D, NPAD], F32, tag="xB")
                pooled = sp.tile([D, ITERS, 128], F32, tag="pooled")

                # ---- iter 0 node embeddings, node-major ----
                pk = cp.tile([128, NPAD // 16], I16, tag="pk")
                nc.sync.dma_start(pk[:], pktloc[:])
                nmA = nmp.tile([128, NWIN, 2 * D], BF16, tag="nmA")
                nc.gpsimd.dma_gather(nmA[:], embs_d[:, :], pk[:], NPAD, NPAD,
                                     2 * D, single_packet=False)

                ht_prev = None
                nm_prev = nmA

                for t in range(ITERS):
                    oh_built = {}

                    def build_oh(b):
                        if b not in oh_built:
                            blk = ohp.tile([128, OH_BATCH, 128], BF16, tag="oh")
                            c0 = OH_BATCH * b
                            if OH_MODE == "tt":
                                nc.vector.tensor_tensor(
                                    blk[:], iotbB,
                                    dsob[:, c0:c0 + OH_BATCH]
                                    .broadcast_to((128, OH_BATCH, 128)),
                                    op=ALU.is_equal)
                            else:
                                for j in range(OH_BATCH):
                                    nc.vector.tensor_scalar(
                                        blk[:, j, :], iotb[:, 0:128],
                                        dso[:, c0 + j:c0 + j + 1], None,
                                        ALU.is_equal)
                            oh_built[b] = blk
                        return oh_built[b]

                    def get_oh(col):
                        if "oh" in skip:
                            col = 0
                        return build_oh(col // OH_BATCH)[:, col % OH_BATCH, :]

                    # MLP chunk helper: runs layers 1-3 + stats for windows
                    # [w0, w1) once their agg is in xA.
                    def mlp_chunk(ci):
                        w0 = ci * MLPCH
                        w1 = min(w0 + MLPCH, NWIN)
                        p0, p1 = w0 * 128, w1 * 128
                        n = p1 - p0
                        # L1: xA->xB, L2/L3: xB->xB in place via PSUM staging
                        if "mlp" not in skip:
                            for l in range(3):
                                xin = xA if l == 0 else xB
                                ps = mlpps.tile([D, 512], F32, tag="mlp")
                                nc.tensor.matmul(
                                    ps[:, 0:n], wm[:, l, :], xin[:, p0:p1],
                                    start=True, stop=True)
                                nc.scalar.activation(
                                    xB[:, p0:p1], ps[:, 0:n], ACT_F.Relu,
                                    bias=bp[:, l:l + 1])
                        xS = xA if "mlp" in skip else xB
                        # per-chunk BN stats over valid (non-pad) columns
                        v0, v1 = p0, min(p1, NPC)
                        if v1 > v0:
                            nc.vector.reduce_sum(
                                stat[:, ci:ci + 1], xS[:, v0:v1], axis=AX)
                            sq = sp.tile([D, 512], F32, tag="sqscr", bufs=2)
                            nc.scalar.activation(
                                sq[:, 0:v1 - v0], xS[:, v0:v1], ACT_F.Square,
                                accum_out=stat2[:, ci:ci + 1])

                    stat = sp.tile([D, NSTAT], F32, tag="stat", bufs=2)
                    stat2 = sp.tile([D, NSTAT], F32, tag="stat2", bufs=2)

                    # ---- aggregation + interleaved MLP ----
                    if t == 0:
                        for w in range(NWIN):
                            agg = aggps.tile([D, 128], F32, tag="agg")
                            if "cnt" in skip:
                                nc.tensor.matmul(
                                    agg[:], nm_prev[:, w, 0:D], epsI[:],
                                    start=True, stop=True)
                            else:
                                slab = cntp.tile([128, VP, 128], BF16,
                                                 tag="cnt")
                                nc.sync.dma_start(slab[:], cntT_d[:, w, :, :])
                                nc.tensor.matmul(
                                    agg[:], nm_prev[:, w, 0:D], epsI[:],
                                    start=True, stop=False)
                                for c in range(VP):
                                    nc.tensor.matmul(
                                        agg[:], emt[:, c, :], slab[:, c, :],
                                        start=False, stop=(c == VP - 1))
                            nc.scalar.activation(
                                xA[:, w * 128:(w + 1) * 128], agg[:],
                                ACT_F.Copy)
                            if (w + 1) % MLPCH == 0 or w == NWIN - 1:
                                mlp_chunk(w // MLPCH)
                    elif "agg" in skip:
                        for w in range(NWIN):
                            agg = aggps.tile([D, 128], F32, tag="agg")
                            nc.tensor.matmul(
                                agg[:], nm_prev[:, w, 0:D], epsI[:],
                                start=True, stop=True)
                            nc.scalar.activation(
                                xA[:, w * 128:(w + 1) * 128], agg[:],
                                ACT_F.Copy)
                            if (w + 1) % MLPCH == 0 or w == NWIN - 1:
                                mlp_chunk(w // MLPCH)
                    else:
                        tab = ht_prev[:].rearrange("(a b) d -> a (b d)", b=2)
                        if "gather" in skip:
                            dummy = cp.tile([128, 128], BF16, tag="dummy")
                            nc.vector.tensor_copy(dummy[:], epsI[:])
                        for g in range(NGRP):
                            NE, NO = ne_g[g], no_g[g]
                            ce, co = cape[g], capo[g]
                            if "gather" not in skip:
                                ixe = ixp.tile([128, NE // 16], I16, tag="ixe")
                                nc.sync.dma_start(
                                    ixe[:],
                                    i23e[:, int(e_base[g]) // 16:int(e_base[g + 1]) // 16])
                                ixo = ixp.tile([128, NO // 16], I16, tag="ixo")
                                nc.sync.dma_start(
                                    ixo[:],
                                    i23o[:, int(o_base[g]) // 16:int(o_base[g + 1]) // 16])
                                gle = gep.tile([128, GWIN * ce, 128], BF16,
                                               tag="ge")
                                nc.gpsimd.dma_gather(gle[:], tab, ixe[:], NE,
                                                     NE, 128,
                                                     single_packet=SINGLE_PACKET,
                                                     queue_num=g % 4)
                                glo = gop.tile([128, GWIN * co, 128], BF16,
                                               tag="go")
                                nc.gpsimd.dma_gather(glo[:], tab, ixo[:], NO,
                                                     NO, 128,
                                                     single_packet=SINGLE_PACKET,
                                                     queue_num=(g + 2) % 4)
                            if OH_PREBUILD and "oh" not in skip:
                                g2 = g + OH_PREBUILD
                                if g2 < NGRP:
                                    for b in range(
                                            int(col_base[g2]) // OH_BATCH,
                                            -(-int(col_base[g2 + 1]) // OH_BATCH)):
                                        build_oh(b)
                            for j in range(GWIN):
                                w = g * GWIN + j
                                agg = aggps.tile([D, 128], F32, tag="agg")
                                if "mm" in skip:
                                    # build the one-hots but skip the matmuls
                                    for c in range(ce):
                                        get_oh(int(col_base[g]) + j * ce + c)
                                    for c in range(co):
                                        get_oh(int(col_base[g]) + GWIN * ce
                                               + j * co + c)
                                    nc.tensor.matmul(
                                        agg[:], nm_prev[:, w, 0:D], epsI[:],
                                        start=True, stop=True)
                                else:
                                    nc.tensor.matmul(
                                        agg[:], nm_prev[:, w, 0:D], epsI[:],
                                        start=True, stop=False)
                                    for c in range(ce):
                                        col = int(col_base[g]) + j * ce + c
                                        lh = (dummy[:, 0:D] if "gather" in skip
                                              else gle[:, j * ce + c, 0:D])
                                        nc.tensor.matmul(
                                            agg[:], lh, get_oh(col),
                                            start=False, stop=False)
                                    for c in range(co):
                                        col = (int(col_base[g]) + GWIN * ce
                                               + j * co + c)
                                        lh = (dummy[:, D:2 * D]
                                              if "gather" in skip
                                              else glo[:, j * co + c, D:2 * D])
                                        nc.tensor.matmul(
                                            agg[:], lh, get_oh(col),
                                            start=False, stop=(c == co - 1))
                                nc.scalar.activation(
                                    xA[:, w * 128:(w + 1) * 128], agg[:],
                                    ACT_F.Copy)
                                if (w + 1) % MLPCH == 0 or w == NWIN - 1:
                                    mlp_chunk(w // MLPCH)

                    # ---- BatchNorm stats + apply (x3 lives in xB) ----
                    x3 = xB
                    st = sp.tile([D, 2], F32, tag="stats")
                    nc.vector.reduce_sum(st[:, 0:1], stat[:], axis=AX)
                    nc.vector.reduce_sum(st[:, 1:2], stat2[:], axis=AX)
                    cin = dram.tile([D, 2], F32, tag="cin")
                    cout = dram.tile([D, 2], F32, tag="cout",
                                     addr_space="Shared",
                                     name=f"bnout{t}_r{rep}")
                    nc.sync.dma_start(cin[:], st[:])
                    if "coll" in skip:
                        nc.sync.dma_start(cout[:], cin[:])
                    else:
                        nc.gpsimd.collective_compute(
                            "AllReduce", ALU.add, replica_groups=rg,
                            ins=[cin.opt()], outs=[cout.opt()])
                    gs = sp.tile([D, 2], F32, tag="gstats")
                    nc.sync.dma_start(gs[:], cout[:])
                    mv = sp.tile([D, 6], F32, tag="mv")
                    nc.vector.tensor_scalar(
                        mv[:, 0:2], gs[:], 1.0 / C["NNODES"], None, ALU.mult)
                    mean = mv[:, 0:1]
                    ex2 = mv[:, 1:2]
                    var = mv[:, 2:3]
                    sd = mv[:, 3:4]
                    scl = mv[:, 4:5]
                    sh = mv[:, 5:6]
                    nc.vector.tensor_tensor(var, mean, mean, op=ALU.mult)
                    nc.vector.tensor_tensor(var, ex2, var, op=ALU.subtract)
                    nc.scalar.activation(sd, var, ACT_F.Sqrt, bias=bp[:, 6:7])
                    nc.vector.reciprocal(sd, sd)
                    nc.vector.tensor_tensor(scl, bp[:, 3:4], sd, op=ALU.mult)
                    nc.vector.tensor_tensor(sh, scl, mean, op=ALU.mult)
                    nc.vector.tensor_tensor(sh, bp[:, 4:5], sh, op=ALU.subtract)
                    # hb = bf16(gamma*(x3-mean)/sd + beta) in one DVE op
                    hb = hx.tile([D, NPAD], BF16, tag="hb")
                    nc.vector.tensor_scalar(hb[:], x3[:], scl, sh, ALU.mult,
                                            op1=ALU.add)

                    # ---- node-major h; pooling; table + AllGather ----
                    nmB = nmp.tile([128, NWIN, D], BF16,
                                   tag=f"nm{(t + 1) % 2}")
                    nc.sync.dma_start_transpose(nmB[:], hb[:])

                    plp = plps.tile([128, D], F32, tag="gpool")
                    for w in range(NWIN):
                        ohg = ohp.tile([128, 128], BF16, tag="ohg")
                        nc.vector.tensor_scalar(
                            ohg[:], iotb[:, 0:128], gof[:, w:w + 1], None,
                            ALU.is_equal)
                        nc.tensor.matmul(
                            plp[:], ohg[:], nmB[:, w, :],
                            start=(w == 0), stop=(w == NWIN - 1))
                    pln = sp.tile([128, D], F32, tag="pln", bufs=2)
                    nc.vector.tensor_copy(pln[:], plp[:])
                    plT = tpps.tile([D, 128], F32, tag="tpT")
                    nc.tensor.transpose(plT[:], pln[:], id128)
                    nc.vector.tensor_copy(pooled[:, t, :], plT[:])

                    if t < ITERS - 1:
                        hloc = dram.tile([NPC, D], BF16, tag="hloc")
                        nfull = (NPC // 128) * 128
                        hl_v = hloc[0:nfull, :].rearrange(
                            "(w p) d -> p w d", p=128)
                        nc.sync.dma_start(hl_v, nmB[:, 0:NPC // 128, :])
                        rem = NPC - nfull
                        if rem:
                            nc.sync.dma_start(
                                hloc[nfull:NPC, :], nmB[0:rem, NPC // 128, :])
                        ht = dram.tile([C["NNODES"], D], BF16, tag="htab",
                                       addr_space="Shared",
                                       name=f"ht{t}_r{rep}")
                        if "coll" in skip:
                            nc.sync.dma_start(ht[0:NPC, :], hloc[:])
                        else:
                            nc.gpsimd.collective_compute(
                                "AllGather", ALU.bypass, replica_groups=rg,
                                ins=[hloc.opt()], outs=[ht.opt()])
                        ht_prev = ht
                    nm_prev = nmB

                # ---- classifier ----
                cls = mlpps.tile([NCLS, 128], F32, tag="mlp")
                for t in range(ITERS):
                    nc.tensor.matmul(
                        cls[:], wc[:, t, :], pooled[:, t, :],
                        start=(t == 0), stop=(t == ITERS - 1))
                lg = sp.tile([NCLS, 128], F32, tag="lg")
                nc.vector.tensor_tensor(lg[:], cls[:], bcm[:], op=ALU.add)
                nc.sync.dma_start(out[:], lg[:])

    nc.compile()
    return nc


_CACHE = {}


def _get_nc(cfg, caps, reps=1, skip=()):
    key = (tuple(sorted(cfg.items())), caps, reps, tuple(sorted(skip)))
    if key not in _CACHE:
        _CACHE[key] = _build(cfg, caps, reps=reps, skip=skip)
    return _CACHE[key]


def kernel(**inputs) -> np.ndarray:
    cfg = _derive(CFG_FULL)
    in_maps, caps, g0s = _prep(
        cfg, inputs["pkt_length"], inputs["src"], inputs["dst"],
        inputs["graph_ids"], inputs["emb"], inputs["eps"],
        inputs["W1"], inputs["b1"], inputs["W2"], inputs["b2"],
        inputs["W3"], inputs["b3"], inputs["gamma"], inputs["beta"],
        inputs["Wc"], inputs["bc"])
    nc = _get_nc(cfg, caps)
    res = bass_utils.run_bass_kernel_spmd(
        nc, in_maps, core_ids=list(range(cfg["P"])))
    NG, NCLS = cfg["NGRAPH"], cfg["NCLS"]

    def unshard(results):
        logits = np.zeros((NG, NCLS), np.float32)
        for k in range(cfg["P"]):
            o = results[k]["logits"]  # [NCLS, 128]
            hi = min(g0s[k] + 128, NG)
            logits[g0s[k]:hi] += o[:, 0:hi - g0s[k]].T
        return logits

    logits = unshard(res.results)
    if not np.any(logits):
        # cold-start collective flake guard: re-execute once
        res = bass_utils.run_bass_kernel_spmd(
            nc, in_maps, core_ids=list(range(cfg["P"])))
        logits = unshard(res.results)
    return logits


# revision 3
# speedup vs baseline: 1.0280x; 1.0280x over previous
"""GIN message-passing classifier on 8 Trainium2 NeuronCores (v3).

Sharding: nodes (and incident edges, partitioned by dst) split contiguously
across 8 cores. Key design vs v2:
  - h table is PAIRED single-bf16: row r = [h[2r] | h[2r+1]] (128 cols,
    256B rows), so the AllGather moves 6.4MB (not 12.8) and indices fit
    int16 with no lo/hi table split.
  - edges are split host-side into even/odd src-parity streams; every
    128-edge chunk is parity-homogeneous, so scatter is ONE matmul per
    chunk (lhsT = gathered cols 0:64 or 64:128).
  - self term (1+eps)*h via identity matmul accumulated into the same
    PSUM as the aggregation (no DVE adds, no feature-major h copy).
  - MLP runs per 512-col chunk interleaved with aggregation; BN stats
    accumulate per chunk; BN apply is one ACT op producing bf16 directly.
  - iter 0 aggregation = embsT @ CNT dense matmuls (CNT edge counts
    precomputed on host).
  - BN stats AllReduced; h table AllGathered (compact).
Host sums per-core partial logits at unshard.
"""

import sys

sys.path.insert(0, "/opt/trn_rl_repo")

import numpy as np

import concourse.bass as bass
import concourse.bacc as bacc
import concourse.mybir as mybir
import concourse.tile as tile
from concourse import bass_utils, library_config

F32 = mybir.dt.float32
BF16 = mybir.dt.bfloat16
I16 = mybir.dt.int16
AX = mybir.AxisListType.X
ALU = mybir.AluOpType
ACT_F = mybir.ActivationFunctionType

OH_BATCH = 8   # one-hot chunks built per DVE op
OH_MODE = "tt"  # "tt": batched tensor_tensor; "ts": per-chunk tensor_scalar
SINGLE_PACKET = False  # dma_gather single_packet flag
OH_PREBUILD = 0  # eagerly emit oh builds for the next N groups
OH_SPLIT = 0  # if >0, every OH_SPLIT-th one-hot block builds on gpsimd
CNT_FP8 = False  # iter-0 CNT slabs in fp8 (hi+lo fp8 emb planes, scale 16)

CFG_FULL = dict(
    NNODES=50000,
    NEDGES=1600000,
    D=64,
    NGRAPH=512,
    NCLS=53,
    VOCAB=3100,
    P=8,
    ITERS=3,
    WINSZ=128,
    GWIN=2,
    BN_EPS=1e-5,
    MLPCH=4,  # windows per MLP chunk
)


def _derive(cfg):
    c = dict(cfg)
    c["NPC"] = c["NNODES"] // c["P"]
    nwr = -(-c["NPC"] // c["WINSZ"])  # ceil
    c["NWIN"] = -(-nwr // c["GWIN"]) * c["GWIN"]
    c["NGRP"] = c["NWIN"] // c["GWIN"]
    c["NPAD"] = c["NWIN"] * c["WINSZ"]
    c["VP"] = -(-c["VOCAB"] // 128)
    return c


def _wrap16(idx):
    n = len(idx)
    assert n % 16 == 0
    arr = np.zeros((16, n // 16), np.int16)
    ar = np.arange(n)
    arr[ar % 16, ar // 16] = idx.astype(np.int16)
    return np.tile(arr, (8, 1))


def _bc_mats(cfg, gids, bc):
    P, NPC, NCLS, NG = cfg["P"], cfg["NPC"], cfg["NCLS"], cfg["NGRAPH"]
    bc = np.asarray(bc, np.float32)
    g0s = [int(gids[k * NPC]) for k in range(P)]
    first = np.searchsorted(gids, np.arange(NG), "left")
    owner = np.minimum(first // NPC, P - 1)
    mats = [np.zeros((NCLS, 128), np.float32) for _ in range(P)]
    for g in range(NG):
        k = int(owner[g])
        s = g - g0s[k]
        if 0 <= s < 128:
            mats[k][:, s] = bc
    return mats


def _prep(cfg, pkt, src, dst, gids, emb, eps, W1, b1, W2, b2, W3, b3, gamma, beta, Wc, bc):
    import ml_dtypes
    P, NPC, WINSZ, NWIN, GWIN, NGRP, D, VP = (
        cfg["P"], cfg["NPC"], cfg["WINSZ"], cfg["NWIN"], cfg["GWIN"],
        cfg["NGRP"], cfg["D"], cfg["VP"],
    )
    NPAD = cfg["NPAD"]
    pkt = np.asarray(pkt); src = np.asarray(src); dst = np.asarray(dst)
    gids = np.asarray(gids)

    k_of = dst // NPC
    per_core = []
    for k in range(P):
        m = k_of == k
        es = src[m]
        el = dst[m] - k * NPC
        win = el // WINSZ
        off = el % WINSZ
        par = (es % 2).astype(np.int64)
        per_core.append((es, win, off, par))

    # per-group static capacities per parity stream (max over cores/windows)
    cape = np.ones(NGRP, np.int64)
    capo = np.ones(NGRP, np.int64)
    for es, win, off, par in per_core:
        for h, caps in ((0, cape), (1, capo)):
            cnt = np.bincount(win[par == h], minlength=NWIN)
            wc_ = -(-cnt // 128)
            gc = wc_.reshape(NGRP, GWIN).max(axis=1)
            np.maximum(caps, gc, out=caps)
    caps_key = (tuple(int(x) for x in cape), tuple(int(x) for x in capo))

    ne_g = GWIN * cape * 128
    no_g = GWIN * capo * 128
    e_base = np.concatenate([[0], np.cumsum(ne_g)]).astype(int)
    o_base = np.concatenate([[0], np.cumsum(no_g)]).astype(int)
    ncols_g = GWIN * (cape + capo)
    col_base = np.concatenate([[0], np.cumsum(ncols_g)]).astype(int)
    NCHUNK = int(col_base[-1])
    NCHUNKB = -(-NCHUNK // OH_BATCH) * OH_BATCH

    emb = np.asarray(emb, np.float32)
    embh = emb.astype(ml_dtypes.bfloat16)  # [VOCAB, D]
    embs_pad = np.zeros((cfg["VOCAB"], 2 * D), ml_dtypes.bfloat16)
    embs_pad[:, 0:D] = embh
    VOCABP = VP * 128
    import concourse.mybir as _mb
    FP8NP = _mb.dt.np(_mb.dt.float8e4)
    if CNT_FP8:
        # two fp8 planes of 16*emb (hi + residual), columns [hi | lo]
        e16 = np.zeros((VOCABP, D), np.float32)
        e16[:cfg["VOCAB"]] = 16.0 * emb
        hi = e16.astype(FP8NP)
        lo = (e16 - hi.astype(np.float32)).astype(FP8NP)
        embsT_full = np.concatenate([hi, lo], axis=1)  # [VOCABP, 2D] fp8
        embsT = np.ascontiguousarray(
            embsT_full.reshape(VP, 128, 2 * D).transpose(1, 0, 2))
    else:
        embsT_full = np.zeros((VOCABP, D), ml_dtypes.bfloat16)
        embsT_full[:cfg["VOCAB"]] = embh
        embsT = np.ascontiguousarray(
            embsT_full.reshape(VP, 128, D).transpose(1, 0, 2))  # [128, VP, D]

    bcm_all = _bc_mats(cfg, gids, bc)
    g0s = []
    in_maps = []
    for k in range(P):
        es, win, off, par = per_core[k]

        i23 = {}
        dso = np.full((NCHUNKB, 128), -1e6, np.float32)
        for h in (0, 1):
            sel = par == h
            w_h, off_h, es_h = win[sel], off[sel], es[sel]
            # sort by (window, src) so each 128-slot chunk reads ascending
            # table addresses (HBM row-buffer locality for the gather)
            order = np.lexsort((es_h, w_h))
            w_h, off_h, es_h = w_h[order], off_h[order], es_h[order]
            cnt = np.bincount(w_h, minlength=NWIN)
            start = np.concatenate([[0], np.cumsum(cnt)])[:-1]
            rank = np.arange(len(w_h)) - start[w_h]
            caps = (cape if h == 0 else capo)
            g = w_h // GWIN
            jw = w_h % GWIN
            capw = caps[g]
            base = (e_base if h == 0 else o_base)[g]
            pos = base + jw * (capw * 128) + rank
            size = int((e_base if h == 0 else o_base)[-1])
            idxs = np.zeros(size, np.int64)
            idxs[pos] = es_h // 2
            i23[h] = idxs
            cb = col_base[g] + (0 if h == 0 else GWIN * cape[g])
            col = cb + jw * capw + rank // 128
            dso[col, rank % 128] = off_h.astype(np.float32)

        def blocks(stream, base_arr):
            out = []
            for g in range(NGRP):
                out.append(_wrap16(stream[base_arr[g]:base_arr[g + 1]]))
            return np.concatenate(out, axis=1)

        i23e = blocks(i23[0], e_base)
        i23o = blocks(i23[1], o_base)

        # CNT matrix for iteration-0 dense aggregation
        m = k_of == k
        cnt0 = np.zeros((VOCABP, NPAD), np.float32)
        np.add.at(cnt0, (pkt[np.asarray(src[m])], np.asarray(dst[m]) - k * NPC), 1.0)
        if CNT_FP8:
            assert cnt0.max() <= 16, "fp8 CNT requires counts <= 16"
            cntT = np.ascontiguousarray(
                cnt0.reshape(VP, 128, NWIN, 128).transpose(1, 2, 0, 3)
            ).astype(FP8NP)  # [128, NWIN, VP, 128] fp8
        else:
            cntT = np.ascontiguousarray(
                cnt0.reshape(VP, 128, NWIN, 128).transpose(1, 2, 0, 3)
            ).astype(ml_dtypes.bfloat16)  # [128, NWIN, VP, 128]

        nloc = np.zeros(NPAD, np.int64)
        nloc[:NPC] = pkt[k * NPC:(k + 1) * NPC]
        pktloc = _wrap16(nloc)

        g0 = int(gids[k * NPC])
        g0s.append(g0)
        gl = gids[k * NPC:(k + 1) * NPC] - g0
        assert gl.max() < 128, "graph span per core exceeds 128 slots"
        gw = np.full(NPAD, -1e6, np.float32)
        gw[:NPC] = gl.astype(np.float32)
        goff = gw.reshape(NWIN, 128).T.copy()  # [128, NWIN]

        consts = np.zeros((128, 640), np.float32)
        consts[:, 0:512] = np.tile(
            np.arange(128, dtype=np.float32)[None, :], (128, 4))
        consts[:, 512:640] = np.eye(128, dtype=np.float32)

        im = {
            "i23e": i23e, "i23o": i23o,
            "pktloc": pktloc,
            "dso": np.ascontiguousarray(dso.T),  # [128, NCHUNKB] f32
            "goff": goff,
            "embs": embs_pad,
            "embsT": embsT,
            "cntT": cntT,
            "Wmlp": np.stack([np.asarray(W1), np.asarray(W2), np.asarray(W3)], 1)
            .astype(np.float32),
            "bpack": np.stack(
                [np.asarray(b1), np.asarray(b2), np.asarray(b3),
                 np.asarray(gamma), np.asarray(beta),
                 np.full(D, 1.0 + float(np.asarray(eps)), np.float32),
                 np.full(D, cfg["BN_EPS"], np.float32),
                 np.zeros(D, np.float32)], 1
            ).astype(np.float32),
            "WcT": np.asarray(Wc, np.float32)
            .reshape(cfg["ITERS"], D, cfg["NCLS"]).transpose(1, 0, 2).copy(),
            "bcmat": bcm_all[k],
            "consts": consts,
        }
        in_maps.append(im)
    return in_maps, caps_key, g0s


def _build(cfg, caps, reps=1, skip=()):
    """skip: subset of {"agg","coll","cnt","oh","mlp"} -- timing ablations
    (results become incorrect; used only to attribute HW time)."""
    C = cfg
    cape, capo = caps
    D, NWIN, GWIN, NGRP, NPC, NPAD, VP = (
        C["D"], C["NWIN"], C["GWIN"], C["NGRP"], C["NPC"], C["NPAD"], C["VP"])
    ITERS = C["ITERS"]
    NCLS = C["NCLS"]
    MLPCH = C["MLPCH"]
    ne_g = [GWIN * cape[g] * 128 for g in range(NGRP)]
    no_g = [GWIN * capo[g] * 128 for g in range(NGRP)]
    e_base = np.concatenate([[0], np.cumsum(ne_g)]).astype(int)
    o_base = np.concatenate([[0], np.cumsum(no_g)]).astype(int)
    ncols_g = [GWIN * (cape[g] + capo[g]) for g in range(NGRP)]
    col_base = np.concatenate([[0], np.cumsum(ncols_g)]).astype(int)
    NCHUNK = int(col_base[-1])
    NCHUNKB = -(-NCHUNK // OH_BATCH) * OH_BATCH

    NSTAT = -(-NWIN // MLPCH)  # MLP/stat chunks per iter

    nc = bacc.Bacc(None, target_bir_lowering=False, debug=False,
                   num_swdge_queues=4)

    i23e = nc.dram_tensor("i23e", [128, int(e_base[-1]) // 16], I16, kind="ExternalInput")
    i23o = nc.dram_tensor("i23o", [128, int(o_base[-1]) // 16], I16, kind="ExternalInput")
    pktloc = nc.dram_tensor("pktloc", [128, NPAD // 16], I16, kind="ExternalInput")
    dso_d = nc.dram_tensor("dso", [128, NCHUNKB], F32, kind="ExternalInput")
    goff = nc.dram_tensor("goff", [128, NWIN], F32, kind="ExternalInput")
    FP8 = mybir.dt.float8e4
    CNTDT = FP8 if CNT_FP8 else BF16
    EMTW = 2 * D if CNT_FP8 else D
    embs_d = nc.dram_tensor("embs", [C["VOCAB"], 2 * D], BF16, kind="ExternalInput")
    embsT_d = nc.dram_tensor("embsT", [128, VP, EMTW], CNTDT, kind="ExternalInput")
    cntT_d = nc.dram_tensor("cntT", [128, NWIN, VP, 128], CNTDT, kind="ExternalInput")
    Wmlp = nc.dram_tensor("Wmlp", [D, 3, D], F32, kind="ExternalInput")
    bpack = nc.dram_tensor("bpack", [D, 8], F32, kind="ExternalInput")
    WcT = nc.dram_tensor("WcT", [D, ITERS, NCLS], F32, kind="ExternalInput")
    bcmat = nc.dram_tensor("bcmat", [NCLS, 128], F32, kind="ExternalInput")
    consts = nc.dram_tensor("consts", [128, 640], F32, kind="ExternalInput")
    out = nc.dram_tensor("logits", [NCLS, 128], F32, kind="ExternalOutput")

    rg = [list(range(C["P"]))]

    with tile.TileContext(nc) as tc:
        with (
            tc.tile_pool(name="const", bufs=1) as cp,
            tc.tile_pool(name="x", bufs=1) as hx,
            tc.tile_pool(name="ge", bufs=2) as gep,
            tc.tile_pool(name="go", bufs=2) as gop,
            tc.tile_pool(name="ix", bufs=2) as ixp,
            tc.tile_pool(name="cnt", bufs=2) as cntp,
            tc.tile_pool(name="oh", bufs=6 + 8 * OH_PREBUILD) as ohp,
            tc.tile_pool(name="nm", bufs=1) as nmp,
            tc.tile_pool(name="small", bufs=1) as sp,
            tc.tile_pool(name="aggps", bufs=4, space="PSUM") as aggps,
            tc.tile_pool(name="mlpps", bufs=2, space="PSUM") as mlpps,
            tc.tile_pool(name="tpps", bufs=1, space="PSUM") as tpps,
            tc.tile_pool(name="plps", bufs=1, space="PSUM") as plps,
            tc.tile_pool(name="dram", bufs=2, space="DRAM") as dram,
        ):
            nc.gpsimd.load_library(library_config.mlp)

            for rep in range(reps):
                # resident constants
                cst = cp.tile([128, 640], F32, tag="cst")
                nc.sync.dma_start(cst[:], consts[:])
                iota4f = cst[:, 0:512]
                id128 = cst[:, 512:640]
                dso = cp.tile([128, NCHUNKB], F32, tag="dso")
                nc.sync.dma_start(dso[:], dso_d[:])
                iotb = cp.tile([128, OH_BATCH * 128], BF16, tag="iotb")
                for i in range(OH_BATCH // 4):
                    nc.vector.tensor_copy(iotb[:, i * 512:(i + 1) * 512], iota4f)
                iotbB = iotb[:].rearrange("p (c j) -> p c j", j=128)
                dsob = cp.tile([128, NCHUNKB], BF16, tag="dsob")
                nc.vector.tensor_copy(dsob[:], dso[:])
                gof = cp.tile([128, NWIN], F32, tag="gof")
                nc.sync.dma_start(gof[:], goff[:])
                emt = cp.tile([128, VP, EMTW], CNTDT, tag="emt")
                nc.sync.dma_start(emt[:], embsT_d[:])
                wm = cp.tile([D, 3, D], F32, tag="wm")
                nc.sync.dma_start(wm[:], Wmlp[:])
                bp = cp.tile([D, 8], F32, tag="bp")
                nc.sync.dma_start(bp[:], bpack[:])
                wc = cp.tile([D, ITERS, NCLS], F32, tag="wc")
                nc.sync.dma_start(wc[:], WcT[:])
                bcm = cp.tile([NCLS, 128], F32, tag="bcm")
                nc.sync.dma_start(bcm[:], bcmat[:])
                # (1+eps) * I as bf16 for the self-term matmul; (1+eps) is
                # replicated to all 128 partitions via two DMA copies.
                ep128 = sp.tile([128, 1], F32, tag="ep128")
                nc.sync.dma_start(ep128[0:D, :], bpack[:, 5:6])
                nc.sync.dma_start(ep128[D:128, :], bpack[0:128 - D, 5:6])
                epsI = cp.tile([128, 128], BF16, tag="epsI")
                nc.vector.tensor_scalar(epsI[:], id128, ep128[:, 0:1], None,
                                        ALU.mult)
                if CNT_FP8:
                    # iter-0 PSUM holds 16*x0 (fp8 planes are 16-scaled), so
                    # the iter-0 self term needs the same 16x factor
                    epsI16 = cp.tile([128, 128], BF16, tag="epsI16")
                    nc.vector.tensor_scalar(epsI16[:], epsI[:], 16.0, None,
                                            ALU.mult)

                xA = hx.tile([D, NPAD], F32, tag="xA")
                xB = hx.tile([D, NPAD], F32, tag="xB")
                pooled = sp.tile([D, ITERS, 128], F32, tag="pooled")

                # ---- iter 0 node embeddings, node-major ----
                pk = cp.tile([128, NPAD // 16], I16, tag="pk")
                nc.sync.dma_start(pk[:], pktloc[:])
                nmA = nmp.tile([128, NWIN, 2 * D], BF16, tag="nmA")
                nc.gpsimd.dma_gather(nmA[:], embs_d[:, :], pk[:], NPAD, NPAD,
                                     2 * D, single_packet=False)

                ht_prev = None
                nm_prev = nmA

                for t in range(ITERS):
                    oh_built = {}

                    def build_oh(b):
                        if b not in oh_built:
                            blk = ohp.tile([128, OH_BATCH, 128], BF16, tag="oh")
                            c0 = OH_BATCH * b
                            if OH_MODE == "tt":
                                eng = (nc.gpsimd if OH_SPLIT
                                       and b % OH_SPLIT == OH_SPLIT - 1
                                       else nc.vector)
                                eng.tensor_tensor(
                                    blk[:], iotbB,
                                    dsob[:, c0:c0 + OH_BATCH]
                                    .broadcast_to((128, OH_BATCH, 128)),
                                    op=ALU.is_equal)
                            else:
                                for j in range(OH_BATCH):
                                    nc.vector.tensor_scalar(
                                        blk[:, j, :], iotb[:, 0:128],
                                        dso[:, c0 + j:c0 + j + 1], None,
                                        ALU.is_equal)
                            oh_built[b] = blk
                        return oh_built[b]

                    def get_oh(col):
                        if "oh" in skip:
                            col = 0
                        return build_oh(col // OH_BATCH)[:, col % OH_BATCH, :]

                    # MLP chunk helper: runs layers 1-3 + stats for windows
                    # [w0, w1) once their agg is in xA.
                    def mlp_chunk(ci):
                        w0 = ci * MLPCH
                        w1 = min(w0 + MLPCH, NWIN)
                        p0, p1 = w0 * 128, w1 * 128
                        n = p1 - p0
                        # L1: xA->xB, L2/L3: xB->xB in place via PSUM staging
                        if "mlp" not in skip:
                            for l in range(3):
                                xin = xA if l == 0 else xB
                                ps = mlpps.tile([D, 512], F32, tag="mlp")
                                nc.tensor.matmul(
                                    ps[:, 0:n], wm[:, l, :], xin[:, p0:p1],
                                    start=True, stop=True)
                                nc.scalar.activation(
                                    xB[:, p0:p1], ps[:, 0:n], ACT_F.Relu,
                                    bias=bp[:, l:l + 1])
                        xS = xA if "mlp" in skip else xB
                        # per-chunk BN stats over valid (non-pad) columns
                        v0, v1 = p0, min(p1, NPC)
                        if v1 > v0:
                            nc.vector.reduce_sum(
                                stat[:, ci:ci + 1], xS[:, v0:v1], axis=AX)
                            sq = sp.tile([D, 512], F32, tag="sqscr", bufs=2)
                            nc.scalar.activation(
                                sq[:, 0:v1 - v0], xS[:, v0:v1], ACT_F.Square,
                                accum_out=stat2[:, ci:ci + 1])

                    stat = sp.tile([D, NSTAT], F32, tag="stat", bufs=2)
                    stat2 = sp.tile([D, NSTAT], F32, tag="stat2", bufs=2)

                    # ---- aggregation + interleaved MLP ----
                    if t == 0:
                        for w in range(NWIN):
                            agg = aggps.tile([D, 128], F32, tag="agg")
                            if "cnt" in skip:
                                nc.tensor.matmul(
                                    agg[:], nm_prev[:, w, 0:D], epsI[:],
                                    start=True, stop=True)
                            else:
                                slab = cntp.tile([128, VP, 128], CNTDT,
                                                 tag="cnt")
                                nc.sync.dma_start(slab[:], cntT_d[:, w, :, :])
                                nc.tensor.matmul(
                                    agg[:], nm_prev[:, w, 0:D],
                                    epsI16[:] if CNT_FP8 else epsI[:],
                                    start=True, stop=False)
                                if CNT_FP8:
                                    for c in range(VP):
                                        nc.tensor.matmul(
                                            agg[:], emt[:, c, 0:D],
                                            slab[:, c, :],
                                            start=False, stop=False)
                                        nc.tensor.matmul(
                                            agg[:], emt[:, c, D:2 * D],
                                            slab[:, c, :],
                                            start=False, stop=(c == VP - 1))
                                else:
                                    for c in range(VP):
                                        nc.tensor.matmul(
                                            agg[:], emt[:, c, :],
                                            slab[:, c, :],
                                            start=False, stop=(c == VP - 1))
                            nc.scalar.activation(
                                xA[:, w * 128:(w + 1) * 128], agg[:],
                                ACT_F.Copy,
                                scale=(1.0 / 16.0) if CNT_FP8 else 1.0)
                            if (w + 1) % MLPCH == 0 or w == NWIN - 1:
                                mlp_chunk(w // MLPCH)
                    elif "agg" in skip:
                        for w in range(NWIN):
                            agg = aggps.tile([D, 128], F32, tag="agg")
                            nc.tensor.matmul(
                                agg[:], nm_prev[:, w, 0:D], epsI[:],
                                start=True, stop=True)
                            nc.scalar.activation(
                                xA[:, w * 128:(w + 1) * 128], agg[:],
                                ACT_F.Copy)
                            if (w + 1) % MLPCH == 0 or w == NWIN - 1:
                                mlp_chunk(w // MLPCH)
                    else:
                        tab = ht_prev[:].rearrange("(a b) d -> a (b d)", b=2)
                        if "gather" in skip:
                            dummy = cp.tile([128, 128], BF16, tag="dummy")
                            nc.vector.tensor_copy(dummy[:], epsI[:])
                        for g in range(NGRP):
                            NE, NO = ne_g[g], no_g[g]
                            ce, co = cape[g], capo[g]
                            if "gather" not in skip:
                                ixe = ixp.tile([128, NE // 16], I16, tag="ixe")
                                nc.sync.dma_start(
                                    ixe[:],
                                    i23e[:, int(e_base[g]) // 16:int(e_base[g + 1]) // 16])
                                ixo = ixp.tile([128, NO // 16], I16, tag="ixo")
                                nc.sync.dma_start(
                                    ixo[:],
                                    i23o[:, int(o_base[g]) // 16:int(o_base[g + 1]) // 16])
                                gle = gep.tile([128, GWIN * ce, 128], BF16,
                                               tag="ge")
                                nc.gpsimd.dma_gather(gle[:], tab, ixe[:], NE,
                                                     NE, 128,
                                                     single_packet=SINGLE_PACKET,
                                                     queue_num=g % 4)
                                glo = gop.tile([128, GWIN * co, 128], BF16,
                                               tag="go")
                                nc.gpsimd.dma_gather(glo[:], tab, ixo[:], NO,
                                                     NO, 128,
                                                     single_packet=SINGLE_PACKET,
                                                     queue_num=(g + 2) % 4)
                            if OH_PREBUILD and "oh" not in skip:
                                g2 = g + OH_PREBUILD
                                if g2 < NGRP:
                                    for b in range(
                                            int(col_base[g2]) // OH_BATCH,
                                            -(-int(col_base[g2 + 1]) // OH_BATCH)):
                                        build_oh(b)
                            for j in range(GWIN):
                                w = g * GWIN + j
                                agg = aggps.tile([D, 128], F32, tag="agg")
                                if "mm" in skip:
                                    # build the one-hots but skip the matmuls
                                    for c in range(ce):
                                        get_oh(int(col_base[g]) + j * ce + c)
                                    for c in range(co):
                                        get_oh(int(col_base[g]) + GWIN * ce
                                               + j * co + c)
                                    nc.tensor.matmul(
                                        agg[:], nm_prev[:, w, 0:D], epsI[:],
                                        start=True, stop=True)
                                else:
                                    nc.tensor.matmul(
                                        agg[:], nm_prev[:, w, 0:D], epsI[:],
                                        start=True, stop=False)
                                    for c in range(ce):
                                        col = int(col_base[g]) + j * ce + c
                                        lh = (dummy[:, 0:D] if "gather" in skip
                                              else gle[:, j * ce + c, 0:D])
                                        nc.tensor.matmul(
                                            agg[:], lh, get_oh(col),
                                            start=False, stop=False)
                                    for c in range(co):
                                        col = (int(col_base[g]) + GWIN * ce
                                               + j * co + c)
                                        lh = (dummy[:, D:2 * D]
                                              if "gather" in skip
                                              else glo[:, j * co + c, D:2 * D])
                                        nc.tensor.matmul(
                                            agg[:], lh, get_oh(col),
                                            start=False, stop=(c == co - 1))
                                nc.scalar.activation(
                                    xA[:, w * 128:(w + 1) * 128], agg[:],
                                    ACT_F.Copy)
                                if (w + 1) % MLPCH == 0 or w == NWIN - 1:
                                    mlp_chunk(w // MLPCH)

                    # ---- BatchNorm stats + apply (x3 lives in xB) ----
                    x3 = xB
                    st = sp.tile([D, 2], F32, tag="stats")
                    nc.vector.reduce_sum(st[:, 0:1], stat[:], axis=AX)
                    nc.vector.reduce_sum(st[:, 1:2], stat2[:], axis=AX)
                    cin = dram.tile([D, 2], F32, tag="cin")
                    cout = dram.tile([D, 2], F32, tag="cout",
                                     addr_space="Shared",
                                     name=f"bnout{t}_r{rep}")
                    nc.sync.dma_start(cin[:], st[:])
                    if "coll" in skip:
                        nc.sync.dma_start(cout[:], cin[:])
                    else:
                        nc.gpsimd.collective_compute(
                            "AllReduce", ALU.add, replica_groups=rg,
                            ins=[cin.opt()], outs=[cout.opt()])
                    gs = sp.tile([D, 2], F32, tag="gstats")
                    nc.sync.dma_start(gs[:], cout[:])
                    mv = sp.tile([D, 6], F32, tag="mv")
                    nc.vector.tensor_scalar(
                        mv[:, 0:2], gs[:], 1.0 / C["NNODES"], None, ALU.mult)
                    mean = mv[:, 0:1]
                    ex2 = mv[:, 1:2]
                    var = mv[:, 2:3]
                    sd = mv[:, 3:4]
                    scl = mv[:, 4:5]
                    sh = mv[:, 5:6]
                    nc.vector.tensor_tensor(var, mean, mean, op=ALU.mult)
                    nc.vector.tensor_tensor(var, ex2, var, op=ALU.subtract)
                    nc.scalar.activation(sd, var, ACT_F.Sqrt, bias=bp[:, 6:7])
                    nc.vector.reciprocal(sd, sd)
                    nc.vector.tensor_tensor(scl, bp[:, 3:4], sd, op=ALU.mult)
                    nc.vector.tensor_tensor(sh, scl, mean, op=ALU.mult)
                    nc.vector.tensor_tensor(sh, bp[:, 4:5], sh, op=ALU.subtract)
                    # hb = bf16(gamma*(x3-mean)/sd + beta) in one DVE op
                    hb = hx.tile([D, NPAD], BF16, tag="hb")
                    nc.vector.tensor_scalar(hb[:], x3[:], scl, sh, ALU.mult,
                                            op1=ALU.add)

                    # ---- node-major h; pooling; table + AllGather ----
                    nmB = nmp.tile([128, NWIN, D], BF16,
                                   tag=f"nm{(t + 1) % 2}")
                    nc.sync.dma_start_transpose(nmB[:], hb[:])

                    plp = plps.tile([128, D], F32, tag="gpool")
                    for w in range(NWIN):
                        ohg = ohp.tile([128, 128], BF16, tag="ohg")
                        nc.vector.tensor_scalar(
                            ohg[:], iotb[:, 0:128], gof[:, w:w + 1], None,
                            ALU.is_equal)
                        nc.tensor.matmul(
                            plp[:], ohg[:], nmB[:, w, :],
                            start=(w == 0), stop=(w == NWIN - 1))
                    pln = sp.tile([128, D], F32, tag="pln", bufs=2)
                    nc.vector.tensor_copy(pln[:], plp[:])
                    plT = tpps.tile([D, 128], F32, tag="tpT")
                    nc.tensor.transpose(plT[:], pln[:], id128)
                    nc.vector.tensor_copy(pooled[:, t, :], plT[:])

                    if t < ITERS - 1:
                        hloc = dram.tile([NPC, D], BF16, tag="hloc")
                        nfull = (NPC // 128) * 128
                        hl_v = hloc[0:nfull, :].rearrange(
                            "(w p) d -> p w d", p=128)
                        nc.sync.dma_start(hl_v, nmB[:, 0:NPC // 128, :])
                        rem = NPC - nfull
                        if rem:
                            nc.sync.dma_start(
                                hloc[nfull:NPC, :], nmB[0:rem, NPC // 128, :])
                        ht = dram.tile([C["NNODES"], D], BF16, tag="htab",
                                       addr_space="Shared",
                                       name=f"ht{t}_r{rep}")
                        if "coll" in skip:
                            nc.sync.dma_start(ht[0:NPC, :], hloc[:])
                        else:
                            nc.gpsimd.collective_compute(
                                "AllGather", ALU.bypass, replica_groups=rg,
                                ins=[hloc.opt()], outs=[ht.opt()])
                        ht_prev = ht
                    nm_prev = nmB

                # ---- classifier ----
                cls = mlpps.tile([NCLS, 128], F32, tag="mlp")
                for t in range(ITERS):
                    nc.tensor.matmul(
                        cls[:], wc[:, t, :], pooled[:, t, :],
                        start=(t == 0), stop=(t == ITERS - 1))
                lg = sp.tile([NCLS, 128], F32, tag="lg")
                nc.vector.tensor_tensor(lg[:], cls[:], bcm[:], op=ALU.add)
                nc.sync.dma_start(out[:], lg[:])

    nc.compile()
    return nc


_CACHE = {}


def _get_nc(cfg, caps, reps=1, skip=()):
    key = (tuple(sorted(cfg.items())), caps, reps, tuple(sorted(skip)),
           OH_BATCH, OH_MODE, SINGLE_PACKET, OH_PREBUILD, OH_SPLIT, CNT_FP8)
    if key not in _CACHE:
        _CACHE[key] = _build(cfg, caps, reps=reps, skip=skip)
    return _CACHE[key]


def kernel(**inputs) -> np.ndarray:
    cfg = _derive(CFG_FULL)
    in_maps, caps, g0s = _prep(
        cfg, inputs["pkt_length"], inputs["src"], inputs["dst"],
        inputs["graph_ids"], inputs["emb"], inputs["eps"],
        inputs["W1"], inputs["b1"], inputs["W2"], inputs["b2"],
        inputs["W3"], inputs["b3"], inputs["gamma"], inputs["beta"],
        inputs["Wc"], inputs["bc"])
    nc = _get_nc(cfg, caps)
    res = bass_utils.run_bass_kernel_spmd(
        nc, in_maps, core_ids=list(range(cfg["P"])))
    NG, NCLS = cfg["NGRAPH"], cfg["NCLS"]

    def unshard(results):
        logits = np.zeros((NG, NCLS), np.float32)
        for k in range(cfg["P"]):
            o = results[k]["logits"]  # [NCLS, 128]
            hi = min(g0s[k] + 128, NG)
            logits[g0s[k]:hi] += o[:, 0:hi - g0s[k]].T
        return logits

    logits = unshard(res.results)
    if not np.any(logits):
        # cold-start collective flake guard: re-execute once
        res = bass_utils.run_bass_kernel_spmd(
            nc, in_maps, core_ids=list(range(cfg["P"])))
        logits = unshard(res.results)
    return logits
